# revision 26
# baseline (speedup 1.0000x reference)
import os
import sys
import types
from contextlib import ExitStack

sys.path.insert(0, "/opt/trn_rl_repo")

import numpy as np
import ml_dtypes

import concourse.bacc as bacc
import concourse.tile as tile
import concourse.mybir as mybir
from concourse import bass_utils
from concourse.bass_utils import run_bass_kernel_spmd

NCORES = 8
B, N, HX, HS = 32, 4096, 128, 1024
F = 512            # HX * R
COLS = 16384       # W columns per core
NB = 32            # 512-col param blocks per core
SPC = B // NCORES  # samples per core
TS = 512           # tokens per block
TB = N // TS

BF16 = ml_dtypes.bfloat16

LAST_EXEC_NS = None
_cached_nc = None


def _ensure_axon_hooks():
    try:
        import antenv.axon_hooks  # noqa: F401
        return
    except Exception:
        pass
    hook = None
    try:
        import trn_agent_boot.trn_boot as tb
        hook = tb._ntff_profile_via_ctypes("/opt/axon/libaxon_pjrt.so")
    except Exception:
        hook = None
    mod = types.ModuleType("antenv.axon_hooks")
    mod.get_axon_ntff_profile_hook = lambda: hook
    sys.modules["antenv.axon_hooks"] = mod
    try:
        bass_utils.upload_artifacts = lambda tmpdir: tmpdir
    except Exception:
        pass


def _build():
    fp32 = mybir.dt.float32
    bf16 = mybir.dt.bfloat16
    f32r = mybir.dt.float32r
    AF = mybir.ActivationFunctionType
    ALU = mybir.AluOpType

    nc = bacc.Bacc("TRN2", target_bir_lowering=False, debug=False,
                   num_devices=NCORES)
    W_d = nc.dram_tensor("W", [NB, 128, 8 * F], bf16, kind="ExternalInput")
    x_d = nc.dram_tensor("x", [SPC, HX, N], bf16, kind="ExternalInput")
    s_d = nc.dram_tensor("s", [128, 8 * B], bf16, kind="ExternalInput")
    b_d = nc.dram_tensor("b", [NB // 4, 128, F], bf16, kind="ExternalInput")
    g_d = nc.dram_tensor("g", [HX, 1], fp32, kind="ExternalInput")
    o_d = nc.dram_tensor("o", [SPC, HX, N], bf16, kind="ExternalOutput")

    with tile.TileContext(nc) as tc, \
         nc.allow_low_precision(reason="bf16 compute; harness gate is 2e-2"):
        with tc.tile_pool(name="pers", bufs=1) as pers, \
             tc.tile_pool(name="xres", bufs=1) as xres, \
             tc.tile_pool(name="dram", bufs=1, space="DRAM") as dram:
            s_t = pers.tile([128, 8 * B], bf16)
            nc.sync.dma_start(s_t[:], s_d[:])
            g_t = pers.tile([HX, 1], fp32)
            nc.sync.dma_start(g_t[:], g_d[:])
            ones_col = pers.tile([128, 1], bf16)
            nc.vector.memset(ones_col[:], 1.0)
            ones_row = pers.tile([1, 128], bf16)
            nc.vector.memset(ones_row[:], 1.0)
            eps_row = pers.tile([1, 1], fp32)
            nc.vector.memset(eps_row[:], 1e-6)

            xts, xss = [], []
            for i in range(SPC):
                xt = xres.tile([HX, N], bf16, name=f"xt{i}")
                nc.scalar.dma_start(xt[:], x_d[i, :, :])
                xts.append(xt)
                xn = xres.tile([HX, N], bf16, name=f"xn{i}")
                xss.append(xn)

            in_b = dram.tile([B, NB // 4, 4, F], bf16)
            out_b = dram.tile([B, COLS], bf16)

            # phase A: params = s @ W + b, interleaved with the full rmsnorm
            # of x (stats + broadcast + scale), which depends only on x and
            # fills the otherwise DMA-bound window. The broadcast matmul and
            # the xs multiply are emitted two blocks late so the PE/DVE
            # never stall on the ACT->DVE stats round trip.
            with tc.tile_pool(name="wp", bufs=3) as wp, \
                 tc.tile_pool(name="bt", bufs=2) as btp, \
                 tc.tile_pool(name="stg", bufs=2) as stg, \
                 tc.tile_pool(name="xsq", bufs=2) as p_xsq, \
                 tc.tile_pool(name="s1", bufs=2) as p_s1, \
                 tc.tile_pool(name="rrt", bufs=12) as p_rrt, \
                 tc.tile_pool(name="psA", bufs=2, space="PSUM") as psA, \
                 tc.tile_pool(name="pss", bufs=2, space="PSUM") as p_pss, \
                 tc.tile_pool(name="psb", bufs=2, space="PSUM") as p_psb:
                rrts = {}

                def stats_front(k):
                    i, tb = k // TB, k % TB
                    xv = xts[i][:, tb * TS:(tb + 1) * TS]
                    xsq = p_xsq.tile([HX, TS], bf16, name="xsq")
                    nc.gpsimd.tensor_tensor(xsq[:], xv, xv, ALU.mult)
                    pss = p_pss.tile([1, TS], fp32, name="pss")
                    nc.tensor.matmul(pss[:], ones_col[:], xsq[:],
                                     start=True, stop=True)
                    s1 = p_s1.tile([1, TS], fp32, name="s1")
                    nc.scalar.activation(s1[:], pss[:], AF.Sqrt,
                                         bias=eps_row[:], scale=1.0 / HX)
                    rrt = p_rrt.tile([1, TS], fp32, name="rrt")
                    nc.vector.reciprocal_approx_fast(rrt[:], s1[:])
                    rr16 = p_rrt.tile([1, TS], bf16, name="rr16")
                    nc.vector.tensor_copy(rr16[:], rrt[:])
                    rrts[k] = rr16

                def stats_back(k):
                    i, tb = k // TB, k % TB
                    xv = xts[i][:, tb * TS:(tb + 1) * TS]
                    psb = p_psb.tile([HX, TS], fp32, name="psb")
                    nc.tensor.matmul(psb[:], ones_row[:], rrts.pop(k)[:],
                                     start=True, stop=True)
                    nc.vector.tensor_tensor(
                        xss[i][:, tb * TS:(tb + 1) * TS], xv, psb[:],
                        ALU.mult)

                # 2048-col superblocks: 4 column blocks run concurrently in
                # distinct 32-col PE array groups, so the DMA-bound window
                # needs 4x fewer PE cycles even when the HAM clock is cold.
                NSB = NB // 4
                for sb in range(NSB + 1):
                    if sb < NSB:
                        wt = wp.tile([128, 4 * 8 * F], bf16)
                        for j in range(4):
                            nc.sync.dma_start(
                                wt[:, j * 8 * F:(j + 1) * 8 * F],
                                W_d[4 * sb + j, :, :])
                        bt = btp.tile([128, F], bf16)
                        nc.scalar.dma_start(bt[:], b_d[sb, :, :])
                        ps = psA.tile([128, F], fp32)
                        for kt in range(8):
                            for j in range(4):
                                nc.tensor.matmul(
                                    ps[32 * j:32 * (j + 1), :],
                                    s_t[:, kt * B:(kt + 1) * B],
                                    wt[:, (j * 8 + kt) * F:
                                          (j * 8 + kt + 1) * F],
                                    start=(kt == 0), stop=(kt == 7),
                                    tile_position=(0, 32 * j),
                                )
                        st = stg.tile([128, F], bf16)
                        nc.vector.tensor_tensor(st[:], ps[:], bt[:], ALU.add)
                        for j in range(4):
                            nc.scalar.dma_start(
                                in_b[:, sb, j, :],
                                st[32 * j:32 * (j + 1), :])
                        for q in range(4):
                            stats_front(4 * sb + q)
                    if sb >= 1:
                        for q in range(4):
                            stats_back(4 * (sb - 1) + q)

            # all-to-all: row 4*src+i on this core <- core src's params for
            # this core's local sample i
            nc.gpsimd.collective_compute(
                "AllToAll", ALU.bypass,
                replica_groups=[list(range(NCORES))],
                ins=[in_b.opt()], outs=[out_b.opt()],
            )

            # phase C: per-sample weight norms, then bmm1/silu/bmm2.
            # All ACT Sqrt ops are emitted before any Silu so the activation
            # table loads exactly twice in the whole kernel.
            with ExitStack() as es:
                def pool(name, bufs, space=None):
                    kw = {"space": space} if space else {}
                    return es.enter_context(
                        tc.tile_pool(name=name, bufs=bufs, **kw))
                p_fc1 = pool("fc1", 2)
                p_fc1g = pool("fc1g", SPC)
                p_fc2 = pool("fc2", 4 * SPC)
                p_sq = pool("sq", 2)
                p_rn = pool("rn", 2 * SPC)
                p_tmp = pool("tmp", 4)
                p_h1 = pool("h1", 2)
                p_ob = pool("ob", 2)
                p_pn = pool("pn", 2, "PSUM")
                p_ph1 = pool("ph1", 2, "PSUM")
                p_ph2 = pool("ph2", 2, "PSUM")

                fc1gs, fc2rs, rn1s, rn2s = [], [], [], []
                for i in range(SPC):
                    fc1r = p_fc1.tile([HX, F], bf16)
                    for src in range(4):
                        r = 4 * src + i
                        nc.sync.dma_start(
                            fc1r[32 * src:32 * (src + 1), :],
                            out_b[r:r + 1, :].rearrange(
                                "o (a f) -> (o a) f", a=32),
                        )
                    fc2r = []
                    for fb in range(4):
                        r = 16 + 4 * fb + i
                        t = p_fc2.tile([128, HX], bf16)
                        nc.sync.dma_start(
                            t[:],
                            out_b[r:r + 1, :].rearrange(
                                "o (p q) -> (o p) q", p=128),
                        )
                        fc2r.append(t)

                    # fc1 col norms over d -> rn1 [f_part, fb]
                    sq1 = p_sq.tile([HX, F], bf16)
                    nc.vector.tensor_tensor(sq1[:], fc1r[:], fc1r[:], ALU.mult)
                    pn1 = p_pn.tile([128, 4], fp32, name="pn")
                    for fb in range(4):
                        nc.tensor.matmul(
                            pn1[:, fb:fb + 1],
                            sq1[:, fb * 128:(fb + 1) * 128],
                            ones_col[:],
                            start=True, stop=True,
                        )
                    n1 = p_tmp.tile([128, 4], fp32)
                    nc.scalar.activation(n1[:], pn1[:], AF.Sqrt)
                    n1m = p_tmp.tile([128, 4], fp32)
                    nc.vector.tensor_scalar_max(n1m[:], n1[:], 1e-12)
                    rn1 = p_rn.tile([128, 4], fp32)
                    nc.vector.reciprocal_approx_fast(rn1[:], n1m[:])

                    # fc2 col norms over f -> rn2 [d_part, 1]
                    sq2 = p_sq.tile([128, F], bf16)
                    for fb in range(4):
                        nc.vector.tensor_tensor(
                            sq2[:, fb * 128:(fb + 1) * 128],
                            fc2r[fb][:], fc2r[fb][:], ALU.mult)
                    pn2 = p_pn.tile([128, 4], fp32, name="pn")
                    for fb in range(4):
                        nc.tensor.matmul(
                            pn2[:, 0:1],
                            sq2[:, fb * 128:(fb + 1) * 128],
                            ones_col[:],
                            start=(fb == 0), stop=(fb == 3),
                        )
                    n2 = p_tmp.tile([128, 1], fp32)
                    nc.scalar.activation(n2[:], pn2[:, 0:1], AF.Sqrt)
                    n2m = p_tmp.tile([128, 1], fp32)
                    nc.vector.tensor_scalar_max(n2m[:], n2[:], 1e-12)
                    rn2 = p_rn.tile([128, 1], fp32)
                    nc.vector.reciprocal_approx_fast(rn2[:], n2m[:])

                    # fold rmsnorm weight g into fc1 rows (per-partition d)
                    fc1g = p_fc1g.tile([HX, F], bf16)
                    nc.vector.tensor_scalar_mul(fc1g[:], fc1r[:], g_t[:])

                    fc1gs.append(fc1g)
                    fc2rs.append(fc2r)
                    rn1s.append(rn1)
                    rn2s.append(rn2)

                for i in range(SPC):
                    fc1g, fc2r = fc1gs[i], fc2rs[i]
                    rn1, rn2 = rn1s[i], rn2s[i]
                    for tb in range(TB):
                        xv = xts[i][:, tb * TS:(tb + 1) * TS]
                        xs = xss[i][:, tb * TS:(tb + 1) * TS]
                        ph2 = p_ph2.tile([HX, TS], fp32)
                        for fb in range(4):
                            ph1 = p_ph1.tile([128, TS], fp32)
                            nc.tensor.matmul(
                                ph1[:],
                                fc1g[:, fb * 128:(fb + 1) * 128],
                                xs,
                                start=True, stop=True,
                            )
                            h1 = p_h1.tile([128, TS], bf16)
                            nc.scalar.activation(h1[:], ph1[:], AF.Silu,
                                                 scale=rn1[:, fb:fb + 1])
                            nc.tensor.matmul(
                                ph2[:], fc2r[fb][:], h1[:],
                                start=(fb == 0), stop=(fb == 3),
                            )
                        ob = p_ob.tile([HX, TS], bf16)
                        nc.vector.scalar_tensor_tensor(
                            ob[:], ph2[:], rn2[:], xv, ALU.mult, ALU.add)
                        nc.sync.dma_start(o_d[i, :, tb * TS:(tb + 1) * TS],
                                          ob[:])
    nc.compile()
    return nc


def _prep_inputs(x, s, W, b, g):
    s_p = np.ascontiguousarray(
        s.T.reshape(8, 128, B).transpose(1, 0, 2).reshape(128, 8 * B)
    ).astype(BF16)
    g_p = np.ascontiguousarray(g.reshape(HX, 1))
    Wb = W.astype(BF16)
    xb = x.astype(BF16)
    bb = b.astype(BF16)
    in_maps = []
    for c in range(NCORES):
        Wc = Wb[:, c * COLS:(c + 1) * COLS]
        Wc = np.ascontiguousarray(
            Wc.reshape(8, 128, NB, F).transpose(2, 1, 0, 3)
              .reshape(NB, 128, 8 * F))
        bc = np.ascontiguousarray(np.broadcast_to(
            bb[c * COLS:(c + 1) * COLS].reshape(NB // 4, 4, 1, F),
            (NB // 4, 4, 32, F)).reshape(NB // 4, 128, F))
        xc = np.ascontiguousarray(
            xb[SPC * c:SPC * (c + 1)].transpose(0, 2, 1))
        in_maps.append({"W": Wc, "x": xc, "s": s_p, "b": bc, "g": g_p})
    return in_maps


def kernel(x, s, W, b, g):
    global LAST_EXEC_NS, _cached_nc
    x = np.asarray(x, dtype=np.float32)
    s = np.asarray(s, dtype=np.float32)
    W = np.asarray(W, dtype=np.float32)
    b = np.asarray(b, dtype=np.float32)
    g = np.asarray(g, dtype=np.float32)

    trace = os.environ.get("KERNEL_TRACE", "0") == "1"
    if trace:
        _ensure_axon_hooks()
    if _cached_nc is None:
        _cached_nc = _build()
    in_maps = _prep_inputs(x, s, W, b, g)
    res = run_bass_kernel_spmd(_cached_nc, in_maps, list(range(NCORES)),
                               trace=trace)
    LAST_EXEC_NS = res.exec_time_ns
    out = np.concatenate(
        [np.asarray(res.results[c]["o"]).astype(np.float32)
         for c in range(NCORES)], axis=0)
    return np.ascontiguousarray(out.transpose(0, 2, 1))


# revision 28
# speedup vs baseline: 1.3110x; 1.3110x over previous
import os
import sys
import types
from contextlib import ExitStack

sys.path.insert(0, "/opt/trn_rl_repo")

import numpy as np
import ml_dtypes

import concourse.bacc as bacc
import concourse.tile as tile
import concourse.mybir as mybir
from concourse import bass_utils
from concourse.bass_utils import run_bass_kernel_spmd

NCORES = 8
B, N, HX, HS = 32, 4096, 128, 1024
F = 512            # HX * R
COLS = 16384       # W columns per core
NB = 32            # 512-col param blocks per core
SPC = B // NCORES  # samples per core
TS = 512           # tokens per block
TB = N // TS

BF16 = ml_dtypes.bfloat16

LAST_EXEC_NS = None
_cached_nc = None


def _ensure_axon_hooks():
    try:
        import antenv.axon_hooks  # noqa: F401
        return
    except Exception:
        pass
    hook = None
    try:
        import trn_agent_boot.trn_boot as tb
        hook = tb._ntff_profile_via_ctypes("/opt/axon/libaxon_pjrt.so")
    except Exception:
        hook = None
    mod = types.ModuleType("antenv.axon_hooks")
    mod.get_axon_ntff_profile_hook = lambda: hook
    sys.modules["antenv.axon_hooks"] = mod
    try:
        bass_utils.upload_artifacts = lambda tmpdir: tmpdir
    except Exception:
        pass


def _build():
    fp32 = mybir.dt.float32
    bf16 = mybir.dt.bfloat16
    f32r = mybir.dt.float32r
    AF = mybir.ActivationFunctionType
    ALU = mybir.AluOpType

    nc = bacc.Bacc("TRN2", target_bir_lowering=False, debug=False,
                   num_devices=NCORES)
    W_d = nc.dram_tensor("W", [NB, 128, 8 * F], bf16, kind="ExternalInput")
    x_d = nc.dram_tensor("x", [SPC, HX, N], bf16, kind="ExternalInput")
    s_d = nc.dram_tensor("s", [128, 8 * B], bf16, kind="ExternalInput")
    b_d = nc.dram_tensor("b", [NB // 4, 128, F], bf16, kind="ExternalInput")
    g_d = nc.dram_tensor("g", [HX, 1], fp32, kind="ExternalInput")
    o_d = nc.dram_tensor("o", [SPC, HX, N], bf16, kind="ExternalOutput")

    with tile.TileContext(nc) as tc, \
         nc.allow_low_precision(reason="bf16 compute; harness gate is 2e-2"):
        with tc.tile_pool(name="pers", bufs=1) as pers, \
             tc.tile_pool(name="xres", bufs=1) as xres, \
             tc.tile_pool(name="dram", bufs=1, space="DRAM") as dram:
            s_t = pers.tile([128, 8 * B], bf16)
            nc.sync.dma_start(s_t[:], s_d[:])
            g_t = pers.tile([HX, 1], fp32)
            nc.sync.dma_start(g_t[:], g_d[:])
            ones_col = pers.tile([128, 1], bf16)
            nc.vector.memset(ones_col[:], 1.0)
            ones_row = pers.tile([1, 128], bf16)
            nc.vector.memset(ones_row[:], 1.0)
            eps_row = pers.tile([1, 1], fp32)
            nc.vector.memset(eps_row[:], 1e-6)

            xts, xss = [], []
            for i in range(SPC):
                xt = xres.tile([HX, N], bf16, name=f"xt{i}")
                nc.sync.dma_start(xt[:], x_d[i, :, :])
                xts.append(xt)
                xn = xres.tile([HX, N], bf16, name=f"xn{i}")
                xss.append(xn)

            in_b = dram.tile([B, NB // 4, 4, F], bf16)
            out_b = dram.tile([B, COLS], bf16)

            # phase A: params = s @ W + b, interleaved with the full rmsnorm
            # of x (stats + broadcast + scale), which depends only on x and
            # fills the otherwise DMA-bound window. The broadcast matmul and
            # the xs multiply are emitted two blocks late so the PE/DVE
            # never stall on the ACT->DVE stats round trip.
            with tc.tile_pool(name="wp", bufs=2) as wp, \
                 tc.tile_pool(name="bt", bufs=2) as btp, \
                 tc.tile_pool(name="stg", bufs=2) as stg, \
                 tc.tile_pool(name="xsq", bufs=2) as p_xsq, \
                 tc.tile_pool(name="s1", bufs=2) as p_s1, \
                 tc.tile_pool(name="rrt", bufs=12) as p_rrt, \
                 tc.tile_pool(name="psA", bufs=2, space="PSUM") as psA, \
                 tc.tile_pool(name="pss", bufs=2, space="PSUM") as p_pss, \
                 tc.tile_pool(name="psb", bufs=2, space="PSUM") as p_psb:
                rrts = {}

                def stats_front(k):
                    i, tb = k // TB, k % TB
                    xv = xts[i][:, tb * TS:(tb + 1) * TS]
                    xsq = p_xsq.tile([HX, TS], bf16, name="xsq")
                    nc.gpsimd.tensor_tensor(xsq[:], xv, xv, ALU.mult)
                    pss = p_pss.tile([1, TS], fp32, name="pss")
                    nc.tensor.matmul(pss[:], ones_col[:], xsq[:],
                                     start=True, stop=True)
                    s1 = p_s1.tile([1, TS], fp32, name="s1")
                    nc.scalar.activation(s1[:], pss[:], AF.Sqrt,
                                         bias=eps_row[:], scale=1.0 / HX)
                    rrt = p_rrt.tile([1, TS], fp32, name="rrt")
                    nc.vector.reciprocal_approx_fast(rrt[:], s1[:])
                    rr16 = p_rrt.tile([1, TS], bf16, name="rr16")
                    nc.vector.tensor_copy(rr16[:], rrt[:])
                    rrts[k] = rr16

                def stats_back(k):
                    i, tb = k // TB, k % TB
                    xv = xts[i][:, tb * TS:(tb + 1) * TS]
                    psb = p_psb.tile([HX, TS], fp32, name="psb")
                    nc.tensor.matmul(psb[:], ones_row[:], rrts.pop(k)[:],
                                     start=True, stop=True)
                    nc.vector.tensor_tensor(
                        xss[i][:, tb * TS:(tb + 1) * TS], xv, psb[:],
                        ALU.mult)

                # 2048-col superblocks: 4 column blocks run concurrently in
                # distinct 32-col PE array groups, so the DMA-bound window
                # needs 4x fewer PE cycles even when the HAM clock is cold.
                NSB = NB // 4
                for sb in range(NSB + 1):
                    if sb < NSB:
                        wt = wp.tile([128, 4 * 8 * F], bf16)
                        for j in range(4):
                            nc.sync.dma_start(
                                wt[:, j * 8 * F:(j + 1) * 8 * F],
                                W_d[4 * sb + j, :, :])
                        bt = btp.tile([128, F], bf16)
                        nc.sync.dma_start(bt[:], b_d[sb, :, :])
                        ps = psA.tile([128, F], fp32)
                        for kt in range(8):
                            for j in range(4):
                                nc.tensor.matmul(
                                    ps[32 * j:32 * (j + 1), :],
                                    s_t[:, kt * B:(kt + 1) * B],
                                    wt[:, (j * 8 + kt) * F:
                                          (j * 8 + kt + 1) * F],
                                    start=(kt == 0), stop=(kt == 7),
                                    tile_position=(0, 32 * j),
                                )
                        st = stg.tile([128, F], bf16)
                        nc.vector.tensor_tensor(st[:], ps[:], bt[:], ALU.add)
                        for j in range(4):
                            nc.sync.dma_start(
                                in_b[:, sb, j, :],
                                st[32 * j:32 * (j + 1), :])
                        for q in range(4):
                            stats_front(4 * sb + q)
                    if sb >= 1:
                        for q in range(4):
                            stats_back(4 * (sb - 1) + q)

            # all-to-all: row 4*src+i on this core <- core src's params for
            # this core's local sample i
            nc.gpsimd.collective_compute(
                "AllToAll", ALU.bypass,
                replica_groups=[list(range(NCORES))],
                ins=[in_b.opt()], outs=[out_b.opt()],
            )

            # phase C: per-sample weight norms, then bmm1/silu/bmm2.
            # All ACT Sqrt ops are emitted before any Silu so the activation
            # table loads exactly twice in the whole kernel.
            with ExitStack() as es:
                def pool(name, bufs, space=None):
                    kw = {"space": space} if space else {}
                    return es.enter_context(
                        tc.tile_pool(name=name, bufs=bufs, **kw))
                p_fc1 = pool("fc1", 2)
                p_fc1g = pool("fc1g", SPC)
                p_fc2 = pool("fc2", 4 * SPC)
                p_sq = pool("sq", 2)
                p_rn = pool("rn", 2 * SPC)
                p_tmp = pool("tmp", 4)
                p_h1 = pool("h1", 2)
                p_ob = pool("ob", 2)
                p_pn = pool("pn", 2, "PSUM")
                p_ph1 = pool("ph1", 2, "PSUM")
                p_ph2 = pool("ph2", 2, "PSUM")

                fc1gs, fc2rs, rn1s, rn2s = [], [], [], []
                for i in range(SPC):
                    fc1r = p_fc1.tile([HX, F], bf16)
                    for src in range(4):
                        r = 4 * src + i
                        nc.sync.dma_start(
                            fc1r[32 * src:32 * (src + 1), :],
                            out_b[r:r + 1, :].rearrange(
                                "o (a f) -> (o a) f", a=32),
                        )
                    fc2r = []
                    for fb in range(4):
                        r = 16 + 4 * fb + i
                        t = p_fc2.tile([128, HX], bf16)
                        nc.sync.dma_start(
                            t[:],
                            out_b[r:r + 1, :].rearrange(
                                "o (p q) -> (o p) q", p=128),
                        )
                        fc2r.append(t)

                    # fc1 col norms over d -> rn1 [f_part, fb]
                    sq1 = p_sq.tile([HX, F], bf16)
                    nc.vector.tensor_tensor(sq1[:], fc1r[:], fc1r[:], ALU.mult)
                    pn1 = p_pn.tile([128, 4], fp32, name="pn")
                    for fb in range(4):
                        nc.tensor.matmul(
                            pn1[:, fb:fb + 1],
                            sq1[:, fb * 128:(fb + 1) * 128],
                            ones_col[:],
                            start=True, stop=True,
                        )
                    n1 = p_tmp.tile([128, 4], fp32)
                    nc.scalar.activation(n1[:], pn1[:], AF.Sqrt)
                    n1m = p_tmp.tile([128, 4], fp32)
                    nc.vector.tensor_scalar_max(n1m[:], n1[:], 1e-12)
                    rn1 = p_rn.tile([128, 4], fp32)
                    nc.vector.reciprocal_approx_fast(rn1[:], n1m[:])

                    # fc2 col norms over f -> rn2 [d_part, 1]
                    sq2 = p_sq.tile([128, F], bf16)
                    for fb in range(4):
                        nc.vector.tensor_tensor(
                            sq2[:, fb * 128:(fb + 1) * 128],
                            fc2r[fb][:], fc2r[fb][:], ALU.mult)
                    pn2 = p_pn.tile([128, 4], fp32, name="pn")
                    for fb in range(4):
                        nc.tensor.matmul(
                            pn2[:, 0:1],
                            sq2[:, fb * 128:(fb + 1) * 128],
                            ones_col[:],
                            start=(fb == 0), stop=(fb == 3),
                        )
                    n2 = p_tmp.tile([128, 1], fp32)
                    nc.scalar.activation(n2[:], pn2[:, 0:1], AF.Sqrt)
                    n2m = p_tmp.tile([128, 1], fp32)
                    nc.vector.tensor_scalar_max(n2m[:], n2[:], 1e-12)
                    rn2 = p_rn.tile([128, 1], fp32)
                    nc.vector.reciprocal_approx_fast(rn2[:], n2m[:])

                    # fold rmsnorm weight g into fc1 rows (per-partition d)
                    fc1g = p_fc1g.tile([HX, F], bf16)
                    nc.vector.tensor_scalar_mul(fc1g[:], fc1r[:], g_t[:])

                    fc1gs.append(fc1g)
                    fc2rs.append(fc2r)
                    rn1s.append(rn1)
                    rn2s.append(rn2)

                for i in range(SPC):
                    fc1g, fc2r = fc1gs[i], fc2rs[i]
                    rn1, rn2 = rn1s[i], rn2s[i]
                    for tp in range(TB // 2):
                        t0 = 2 * tp * TS
                        xs2 = xss[i][:, t0:t0 + 2 * TS]
                        ph2a = p_ph2.tile([HX, TS], fp32, name="ph2")
                        ph2b = p_ph2.tile([HX, TS], fp32, name="ph2")
                        for fb in range(4):
                            fcs = fc1g[:, fb * 128:(fb + 1) * 128]
                            ph1 = p_ph1.tile([128, 2 * TS], fp32)
                            nc.tensor.matmul(
                                ph1[:, 0:TS], fcs, xs2[:, 0:TS],
                                start=True, stop=True,
                            )
                            nc.tensor.matmul(
                                ph1[:, TS:2 * TS], fcs, xs2[:, TS:2 * TS],
                                start=True, stop=True,
                            )
                            # one silu over both token blocks: same fb, so
                            # the per-partition rn1 scale is shared
                            h1 = p_h1.tile([128, 2 * TS], bf16)
                            nc.scalar.activation(h1[:], ph1[:], AF.Silu,
                                                 scale=rn1[:, fb:fb + 1])
                            nc.tensor.matmul(
                                ph2a[:], fc2r[fb][:], h1[:, 0:TS],
                                start=(fb == 0), stop=(fb == 3),
                            )
                            nc.tensor.matmul(
                                ph2b[:], fc2r[fb][:], h1[:, TS:2 * TS],
                                start=(fb == 0), stop=(fb == 3),
                            )
                        for half, ph2 in ((0, ph2a), (1, ph2b)):
                            sl = slice(t0 + half * TS, t0 + (half + 1) * TS)
                            ob = p_ob.tile([HX, TS], bf16)
                            nc.vector.scalar_tensor_tensor(
                                ob[:], ph2[:], rn2[:], xts[i][:, sl],
                                ALU.mult, ALU.add)
                            nc.sync.dma_start(o_d[i, :, sl], ob[:])
    nc.compile()
    return nc


def _prep_inputs(x, s, W, b, g):
    s_p = np.ascontiguousarray(
        s.T.reshape(8, 128, B).transpose(1, 0, 2).reshape(128, 8 * B)
    ).astype(BF16)
    g_p = np.ascontiguousarray(g.reshape(HX, 1))
    Wb = W.astype(BF16)
    xb = x.astype(BF16)
    bb = b.astype(BF16)
    in_maps = []
    for c in range(NCORES):
        Wc = Wb[:, c * COLS:(c + 1) * COLS]
        Wc = np.ascontiguousarray(
            Wc.reshape(8, 128, NB, F).transpose(2, 1, 0, 3)
              .reshape(NB, 128, 8 * F))
        bc = np.ascontiguousarray(np.broadcast_to(
            bb[c * COLS:(c + 1) * COLS].reshape(NB // 4, 4, 1, F),
            (NB // 4, 4, 32, F)).reshape(NB // 4, 128, F))
        xc = np.ascontiguousarray(
            xb[SPC * c:SPC * (c + 1)].transpose(0, 2, 1))
        in_maps.append({"W": Wc, "x": xc, "s": s_p, "b": bc, "g": g_p})
    return in_maps


def kernel(x, s, W, b, g):
    global LAST_EXEC_NS, _cached_nc
    x = np.asarray(x, dtype=np.float32)
    s = np.asarray(s, dtype=np.float32)
    W = np.asarray(W, dtype=np.float32)
    b = np.asarray(b, dtype=np.float32)
    g = np.asarray(g, dtype=np.float32)

    trace = os.environ.get("KERNEL_TRACE", "0") == "1"
    if trace:
        _ensure_axon_hooks()
    if _cached_nc is None:
        _cached_nc = _build()
    in_maps = _prep_inputs(x, s, W, b, g)
    res = run_bass_kernel_spmd(_cached_nc, in_maps, list(range(NCORES)),
                               trace=trace)
    LAST_EXEC_NS = res.exec_time_ns
    out = np.concatenate(
        [np.asarray(res.results[c]["o"]).astype(np.float32)
         for c in range(NCORES)], axis=0)
    return np.ascontiguousarray(out.transpose(0, 2, 1))


# revision 30
# speedup vs baseline: 1.3645x; 1.0409x over previous
import os
import sys
import types
from contextlib import ExitStack

sys.path.insert(0, "/opt/trn_rl_repo")

import numpy as np
import ml_dtypes

import concourse.bacc as bacc
import concourse.tile as tile
import concourse.mybir as mybir
from concourse import bass_utils
from concourse.bass_utils import run_bass_kernel_spmd

NCORES = 8
B, N, HX, HS = 32, 4096, 128, 1024
F = 512            # HX * R
COLS = 16384       # W columns per core
NB = 32            # 512-col param blocks per core
SPC = B // NCORES  # samples per core
TS = 512           # tokens per block
TB = N // TS

BF16 = ml_dtypes.bfloat16

LAST_EXEC_NS = None
_cached_nc = None


def _ensure_axon_hooks():
    try:
        import antenv.axon_hooks  # noqa: F401
        return
    except Exception:
        pass
    hook = None
    try:
        import trn_agent_boot.trn_boot as tb
        hook = tb._ntff_profile_via_ctypes("/opt/axon/libaxon_pjrt.so")
    except Exception:
        hook = None
    mod = types.ModuleType("antenv.axon_hooks")
    mod.get_axon_ntff_profile_hook = lambda: hook
    sys.modules["antenv.axon_hooks"] = mod
    try:
        bass_utils.upload_artifacts = lambda tmpdir: tmpdir
    except Exception:
        pass


def _build():
    fp32 = mybir.dt.float32
    bf16 = mybir.dt.bfloat16
    f32r = mybir.dt.float32r
    AF = mybir.ActivationFunctionType
    ALU = mybir.AluOpType

    nc = bacc.Bacc("TRN2", target_bir_lowering=False, debug=False,
                   num_devices=NCORES)
    W_d = nc.dram_tensor("W", [NB, 128, 8 * F], bf16, kind="ExternalInput")
    x_d = nc.dram_tensor("x", [SPC, HX, N], bf16, kind="ExternalInput")
    s_d = nc.dram_tensor("s", [128, 8 * B], bf16, kind="ExternalInput")
    b_d = nc.dram_tensor("b", [NB // 4, 128, F], bf16, kind="ExternalInput")
    g_d = nc.dram_tensor("g", [HX, 1], fp32, kind="ExternalInput")
    o_d = nc.dram_tensor("o", [SPC, HX, N], bf16, kind="ExternalOutput")

    with tile.TileContext(nc) as tc, \
         nc.allow_low_precision(reason="bf16 compute; harness gate is 2e-2"):
        with tc.tile_pool(name="pers", bufs=1) as pers, \
             tc.tile_pool(name="xres", bufs=1) as xres, \
             tc.tile_pool(name="dram", bufs=1, space="DRAM") as dram:
            s_t = pers.tile([128, 8 * B], bf16)
            nc.sync.dma_start(s_t[:], s_d[:])
            g_t = pers.tile([HX, 1], fp32)
            nc.sync.dma_start(g_t[:], g_d[:])
            ones_col = pers.tile([128, 1], bf16)
            nc.vector.memset(ones_col[:], 1.0)
            ones_row = pers.tile([1, 128], bf16)
            nc.vector.memset(ones_row[:], 1.0)
            eps_row = pers.tile([1, 1], fp32)
            nc.vector.memset(eps_row[:], 1e-6)

            xts, xss = [], []
            for i in range(SPC):
                xt = xres.tile([HX, N], bf16, name=f"xt{i}")
                nc.sync.dma_start(xt[:], x_d[i, :, :])
                xts.append(xt)
                xn = xres.tile([HX, N], bf16, name=f"xn{i}")
                xss.append(xn)

            in_b = dram.tile([B, NB // 4, 4, F], bf16)
            out_b = dram.tile([B, COLS], bf16)

            # phase A: params = s @ W + b, interleaved with the full rmsnorm
            # of x (stats + broadcast + scale), which depends only on x and
            # fills the otherwise DMA-bound window. The broadcast matmul and
            # the xs multiply are emitted two blocks late so the PE/DVE
            # never stall on the ACT->DVE stats round trip.
            with tc.tile_pool(name="wp", bufs=3) as wp, \
                 tc.tile_pool(name="bt", bufs=2) as btp, \
                 tc.tile_pool(name="stg", bufs=2) as stg, \
                 tc.tile_pool(name="xsq", bufs=2) as p_xsq, \
                 tc.tile_pool(name="s1", bufs=2) as p_s1, \
                 tc.tile_pool(name="rrt", bufs=12) as p_rrt, \
                 tc.tile_pool(name="psA", bufs=2, space="PSUM") as psA, \
                 tc.tile_pool(name="pss", bufs=2, space="PSUM") as p_pss, \
                 tc.tile_pool(name="psb", bufs=2, space="PSUM") as p_psb:
                rrts = {}

                def stats_front(k):
                    i, tb = k // TB, k % TB
                    xv = xts[i][:, tb * TS:(tb + 1) * TS]
                    xsq = p_xsq.tile([HX, TS], bf16, name="xsq")
                    nc.gpsimd.tensor_tensor(xsq[:], xv, xv, ALU.mult)
                    pss = p_pss.tile([1, TS], fp32, name="pss")
                    nc.tensor.matmul(pss[:], ones_col[:], xsq[:],
                                     start=True, stop=True)
                    s1 = p_s1.tile([1, TS], fp32, name="s1")
                    nc.scalar.activation(s1[:], pss[:], AF.Sqrt,
                                         bias=eps_row[:], scale=1.0 / HX)
                    rrt = p_rrt.tile([1, TS], fp32, name="rrt")
                    nc.vector.reciprocal_approx_fast(rrt[:], s1[:])
                    rr16 = p_rrt.tile([1, TS], bf16, name="rr16")
                    nc.vector.tensor_copy(rr16[:], rrt[:])
                    rrts[k] = rr16

                def stats_back(k):
                    i, tb = k // TB, k % TB
                    xv = xts[i][:, tb * TS:(tb + 1) * TS]
                    psb = p_psb.tile([HX, TS], fp32, name="psb")
                    nc.tensor.matmul(psb[:], ones_row[:], rrts.pop(k)[:],
                                     start=True, stop=True)
                    nc.vector.tensor_tensor(
                        xss[i][:, tb * TS:(tb + 1) * TS], xv, psb[:],
                        ALU.mult)

                # 2048-col superblocks: 4 column blocks run concurrently in
                # distinct 32-col PE array groups, so the DMA-bound window
                # needs 4x fewer PE cycles even when the HAM clock is cold.
                NSB = NB // 4
                for sb in range(NSB + 1):
                    if sb < NSB:
                        wt = wp.tile([128, 4 * 8 * F], bf16)
                        for j in range(4):
                            nc.sync.dma_start(
                                wt[:, j * 8 * F:(j + 1) * 8 * F],
                                W_d[4 * sb + j, :, :])
                        bt = btp.tile([128, F], bf16)
                        nc.sync.dma_start(bt[:], b_d[sb, :, :])
                        ps = psA.tile([128, F], fp32)
                        for kt in range(8):
                            for j in range(4):
                                nc.tensor.matmul(
                                    ps[32 * j:32 * (j + 1), :],
                                    s_t[:, kt * B:(kt + 1) * B],
                                    wt[:, (j * 8 + kt) * F:
                                          (j * 8 + kt + 1) * F],
                                    start=(kt == 0), stop=(kt == 7),
                                    tile_position=(0, 32 * j),
                                )
                        st = stg.tile([128, F], bf16)
                        nc.vector.tensor_tensor(st[:], ps[:], bt[:], ALU.add)
                        for j in range(4):
                            nc.sync.dma_start(
                                in_b[:, sb, j, :],
                                st[32 * j:32 * (j + 1), :])
                        for q in range(4):
                            stats_front(4 * sb + q)
                    if sb >= 1:
                        for q in range(4):
                            stats_back(4 * (sb - 1) + q)

            # all-to-all: row 4*src+i on this core <- core src's params for
            # this core's local sample i
            nc.gpsimd.collective_compute(
                "AllToAll", ALU.bypass,
                replica_groups=[list(range(NCORES))],
                ins=[in_b.opt()], outs=[out_b.opt()],
            )

            # phase C: per-sample weight norms, then bmm1/silu/bmm2.
            # All ACT Sqrt ops are emitted before any Silu so the activation
            # table loads exactly twice in the whole kernel.
            with ExitStack() as es:
                def pool(name, bufs, space=None):
                    kw = {"space": space} if space else {}
                    return es.enter_context(
                        tc.tile_pool(name=name, bufs=bufs, **kw))
                p_fc1 = pool("fc1", 2)
                p_fc1g = pool("fc1g", SPC)
                p_fc2 = pool("fc2", 4 * SPC)
                p_sq = pool("sq", 2)
                p_rn = pool("rn", 2 * SPC)
                p_tmp = pool("tmp", 4)
                p_h1 = pool("h1", 2)
                p_ob = pool("ob", 2)
                p_pn = pool("pn", 2, "PSUM")
                p_ph1 = pool("ph1", 2, "PSUM")
                p_ph2 = pool("ph2", 2, "PSUM")

                fc1gs, fc2rs, rn1s, rn2s = [], [], [], []
                for i in range(SPC):
                    fc1r = p_fc1.tile([HX, F], bf16)
                    for src in range(4):
                        r = 4 * src + i
                        nc.sync.dma_start(
                            fc1r[32 * src:32 * (src + 1), :],
                            out_b[r:r + 1, :].rearrange(
                                "o (a f) -> (o a) f", a=32),
                        )
                    fc2r = []
                    for fb in range(4):
                        r = 16 + 4 * fb + i
                        t = p_fc2.tile([128, HX], bf16)
                        nc.sync.dma_start(
                            t[:],
                            out_b[r:r + 1, :].rearrange(
                                "o (p q) -> (o p) q", p=128),
                        )
                        fc2r.append(t)

                    # fc1 col norms over d -> rn1 [f_part, fb]
                    sq1 = p_sq.tile([HX, F], bf16)
                    nc.vector.tensor_tensor(sq1[:], fc1r[:], fc1r[:], ALU.mult)
                    pn1 = p_pn.tile([128, 4], fp32, name="pn")
                    for fb in range(4):
                        nc.tensor.matmul(
                            pn1[:, fb:fb + 1],
                            sq1[:, fb * 128:(fb + 1) * 128],
                            ones_col[:],
                            start=True, stop=True,
                        )
                    n1 = p_tmp.tile([128, 4], fp32)
                    nc.scalar.activation(n1[:], pn1[:], AF.Sqrt)
                    n1m = p_tmp.tile([128, 4], fp32)
                    nc.vector.tensor_scalar_max(n1m[:], n1[:], 1e-12)
                    rn1 = p_rn.tile([128, 4], fp32)
                    nc.vector.reciprocal_approx_fast(rn1[:], n1m[:])

                    # fc2 col norms over f -> rn2 [d_part, 1]
                    sq2 = p_sq.tile([128, F], bf16)
                    for fb in range(4):
                        nc.vector.tensor_tensor(
                            sq2[:, fb * 128:(fb + 1) * 128],
                            fc2r[fb][:], fc2r[fb][:], ALU.mult)
                    pn2 = p_pn.tile([128, 4], fp32, name="pn")
                    for fb in range(4):
                        nc.tensor.matmul(
                            pn2[:, 0:1],
                            sq2[:, fb * 128:(fb + 1) * 128],
                            ones_col[:],
                            start=(fb == 0), stop=(fb == 3),
                        )
                    n2 = p_tmp.tile([128, 1], fp32)
                    nc.scalar.activation(n2[:], pn2[:, 0:1], AF.Sqrt)
                    n2m = p_tmp.tile([128, 1], fp32)
                    nc.vector.tensor_scalar_max(n2m[:], n2[:], 1e-12)
                    rn2 = p_rn.tile([128, 1], fp32)
                    nc.vector.reciprocal_approx_fast(rn2[:], n2m[:])

                    # fold rmsnorm weight g into fc1 rows (per-partition d)
                    fc1g = p_fc1g.tile([HX, F], bf16)
                    nc.vector.tensor_scalar_mul(fc1g[:], fc1r[:], g_t[:])

                    fc1gs.append(fc1g)
                    fc2rs.append(fc2r)
                    rn1s.append(rn1)
                    rn2s.append(rn2)

                for i in range(SPC):
                    fc1g, fc2r = fc1gs[i], fc2rs[i]
                    rn1, rn2 = rn1s[i], rn2s[i]
                    for tb in range(TB):
                        xv = xts[i][:, tb * TS:(tb + 1) * TS]
                        xs = xss[i][:, tb * TS:(tb + 1) * TS]
                        ph2 = p_ph2.tile([HX, TS], fp32)
                        for fb in range(4):
                            ph1 = p_ph1.tile([128, TS], fp32)
                            nc.tensor.matmul(
                                ph1[:],
                                fc1g[:, fb * 128:(fb + 1) * 128],
                                xs,
                                start=True, stop=True,
                            )
                            h1 = p_h1.tile([128, TS], bf16)
                            nc.scalar.activation(h1[:], ph1[:], AF.Silu,
                                                 scale=rn1[:, fb:fb + 1])
                            nc.tensor.matmul(
                                ph2[:], fc2r[fb][:], h1[:],
                                start=(fb == 0), stop=(fb == 3),
                            )
                        ob = p_ob.tile([HX, TS], bf16)
                        nc.vector.scalar_tensor_tensor(
                            ob[:], ph2[:], rn2[:], xv, ALU.mult, ALU.add)
                        nc.sync.dma_start(o_d[i, :, tb * TS:(tb + 1) * TS],
                                          ob[:])
    nc.compile()
    return nc


def _prep_inputs(x, s, W, b, g):
    s_p = np.ascontiguousarray(
        s.T.reshape(8, 128, B).transpose(1, 0, 2).reshape(128, 8 * B)
    ).astype(BF16)
    g_p = np.ascontiguousarray(g.reshape(HX, 1))
    Wb = W.astype(BF16)
    xb = x.astype(BF16)
    bb = b.astype(BF16)
    in_maps = []
    for c in range(NCORES):
        Wc = Wb[:, c * COLS:(c + 1) * COLS]
        Wc = np.ascontiguousarray(
            Wc.reshape(8, 128, NB, F).transpose(2, 1, 0, 3)
              .reshape(NB, 128, 8 * F))
        bc = np.ascontiguousarray(np.broadcast_to(
            bb[c * COLS:(c + 1) * COLS].reshape(NB // 4, 4, 1, F),
            (NB // 4, 4, 32, F)).reshape(NB // 4, 128, F))
        xc = np.ascontiguousarray(
            xb[SPC * c:SPC * (c + 1)].transpose(0, 2, 1))
        in_maps.append({"W": Wc, "x": xc, "s": s_p, "b": bc, "g": g_p})
    return in_maps


def kernel(x, s, W, b, g):
    global LAST_EXEC_NS, _cached_nc
    x = np.asarray(x, dtype=np.float32)
    s = np.asarray(s, dtype=np.float32)
    W = np.asarray(W, dtype=np.float32)
    b = np.asarray(b, dtype=np.float32)
    g = np.asarray(g, dtype=np.float32)

    trace = os.environ.get("KERNEL_TRACE", "0") == "1"
    if trace:
        _ensure_axon_hooks()
    if _cached_nc is None:
        _cached_nc = _build()
    in_maps = _prep_inputs(x, s, W, b, g)
    res = run_bass_kernel_spmd(_cached_nc, in_maps, list(range(NCORES)),
                               trace=trace)
    LAST_EXEC_NS = res.exec_time_ns
    out = np.concatenate(
        [np.asarray(res.results[c]["o"]).astype(np.float32)
         for c in range(NCORES)], axis=0)
    return np.ascontiguousarray(out.transpose(0, 2, 1))


# revision 32
# speedup vs baseline: 1.4086x; 1.0323x over previous
import os
import sys
import types
from contextlib import ExitStack

sys.path.insert(0, "/opt/trn_rl_repo")

import numpy as np
import ml_dtypes

import concourse.bacc as bacc
import concourse.tile as tile
import concourse.mybir as mybir
from concourse import bass_utils
from concourse.bass_utils import run_bass_kernel_spmd

NCORES = 8
B, N, HX, HS = 32, 4096, 128, 1024
F = 512            # HX * R
COLS = 16384       # W columns per core
NB = 32            # 512-col param blocks per core
SPC = B // NCORES  # samples per core
TS = 512           # tokens per block
TB = N // TS

BF16 = ml_dtypes.bfloat16

LAST_EXEC_NS = None
_cached_nc = None


def _ensure_axon_hooks():
    try:
        import antenv.axon_hooks  # noqa: F401
        return
    except Exception:
        pass
    hook = None
    try:
        import trn_agent_boot.trn_boot as tb
        hook = tb._ntff_profile_via_ctypes("/opt/axon/libaxon_pjrt.so")
    except Exception:
        hook = None
    mod = types.ModuleType("antenv.axon_hooks")
    mod.get_axon_ntff_profile_hook = lambda: hook
    sys.modules["antenv.axon_hooks"] = mod
    try:
        bass_utils.upload_artifacts = lambda tmpdir: tmpdir
    except Exception:
        pass


def _build():
    fp32 = mybir.dt.float32
    bf16 = mybir.dt.bfloat16
    f32r = mybir.dt.float32r
    AF = mybir.ActivationFunctionType
    ALU = mybir.AluOpType

    nc = bacc.Bacc("TRN2", target_bir_lowering=False, debug=False,
                   num_devices=NCORES)
    W_d = nc.dram_tensor("W", [NB, 128, 8 * F], bf16, kind="ExternalInput")
    x_d = nc.dram_tensor("x", [SPC, HX, N], bf16, kind="ExternalInput")
    s_d = nc.dram_tensor("s", [128, 8 * B], bf16, kind="ExternalInput")
    b_d = nc.dram_tensor("b", [NB // 4, 128, F], bf16, kind="ExternalInput")
    g_d = nc.dram_tensor("g", [HX, 1], fp32, kind="ExternalInput")
    o_d = nc.dram_tensor("o", [SPC, HX, N], bf16, kind="ExternalOutput")

    with tile.TileContext(nc) as tc, \
         nc.allow_low_precision(reason="bf16 compute; harness gate is 2e-2"):
        with tc.tile_pool(name="pers", bufs=1) as pers, \
             tc.tile_pool(name="xres", bufs=1) as xres, \
             tc.tile_pool(name="dram", bufs=1, space="DRAM") as dram:
            s_t = pers.tile([128, 8 * B], bf16)
            nc.sync.dma_start(s_t[:], s_d[:])
            g_t = pers.tile([HX, 1], fp32)
            nc.sync.dma_start(g_t[:], g_d[:])
            ones_col = pers.tile([128, 1], bf16)
            nc.vector.memset(ones_col[:], 1.0)
            ones_row = pers.tile([1, 128], bf16)
            nc.vector.memset(ones_row[:], 1.0)
            eps_row = pers.tile([1, 1], fp32)
            nc.vector.memset(eps_row[:], 1e-6)

            xts, xss = [], []
            for i in range(SPC):
                xt = xres.tile([HX, N], bf16, name=f"xt{i}")
                nc.sync.dma_start(xt[:], x_d[i, :, :])
                xts.append(xt)
                xn = xres.tile([HX, N], bf16, name=f"xn{i}")
                xss.append(xn)

            in_b = dram.tile([B, NB // 4, 4, F], bf16)
            out_b = dram.tile([B, COLS], bf16)

            # phase A: params = s @ W + b, interleaved with the full rmsnorm
            # of x (stats + broadcast + scale), which depends only on x and
            # fills the otherwise DMA-bound window. The broadcast matmul and
            # the xs multiply are emitted two blocks late so the PE/DVE
            # never stall on the ACT->DVE stats round trip.
            with tc.tile_pool(name="wp", bufs=2) as wp, \
                 tc.tile_pool(name="bt", bufs=2) as btp, \
                 tc.tile_pool(name="stg", bufs=2) as stg, \
                 tc.tile_pool(name="xsq", bufs=2) as p_xsq, \
                 tc.tile_pool(name="s1", bufs=2) as p_s1, \
                 tc.tile_pool(name="rrt", bufs=12) as p_rrt, \
                 tc.tile_pool(name="psA", bufs=2, space="PSUM") as psA, \
                 tc.tile_pool(name="pss", bufs=2, space="PSUM") as p_pss, \
                 tc.tile_pool(name="psb", bufs=2, space="PSUM") as p_psb:
                rrts = {}

                def stats_front(k):
                    i, tb = k // TB, k % TB
                    xv = xts[i][:, tb * TS:(tb + 1) * TS]
                    xsq = p_xsq.tile([HX, TS], bf16, name="xsq")
                    nc.gpsimd.tensor_tensor(xsq[:], xv, xv, ALU.mult)
                    pss = p_pss.tile([1, TS], fp32, name="pss")
                    nc.tensor.matmul(pss[:], ones_col[:], xsq[:],
                                     start=True, stop=True)
                    s1 = p_s1.tile([1, TS], fp32, name="s1")
                    nc.scalar.activation(s1[:], pss[:], AF.Sqrt,
                                         bias=eps_row[:], scale=1.0 / HX)
                    rrt = p_rrt.tile([1, TS], fp32, name="rrt")
                    nc.vector.reciprocal_approx_fast(rrt[:], s1[:])
                    rr16 = p_rrt.tile([1, TS], bf16, name="rr16")
                    nc.vector.tensor_copy(rr16[:], rrt[:])
                    rrts[k] = rr16

                def stats_back(k):
                    i, tb = k // TB, k % TB
                    xv = xts[i][:, tb * TS:(tb + 1) * TS]
                    psb = p_psb.tile([HX, TS], fp32, name="psb")
                    nc.tensor.matmul(psb[:], ones_row[:], rrts.pop(k)[:],
                                     start=True, stop=True)
                    nc.vector.tensor_tensor(
                        xss[i][:, tb * TS:(tb + 1) * TS], xv, psb[:],
                        ALU.mult)

                # 2048-col superblocks: 4 column blocks run concurrently in
                # distinct 32-col PE array groups, so the DMA-bound window
                # needs 4x fewer PE cycles even when the HAM clock is cold.
                NSB = NB // 4
                for sb in range(NSB + 1):
                    if sb < NSB:
                        wt = wp.tile([128, 4 * 8 * F], bf16)
                        for j in range(4):
                            nc.sync.dma_start(
                                wt[:, j * 8 * F:(j + 1) * 8 * F],
                                W_d[4 * sb + j, :, :])
                        bt = btp.tile([128, F], bf16)
                        nc.sync.dma_start(bt[:], b_d[sb, :, :])
                        ps = psA.tile([128, F], fp32)
                        for kt in range(8):
                            for j in range(4):
                                nc.tensor.matmul(
                                    ps[32 * j:32 * (j + 1), :],
                                    s_t[:, kt * B:(kt + 1) * B],
                                    wt[:, (j * 8 + kt) * F:
                                          (j * 8 + kt + 1) * F],
                                    start=(kt == 0), stop=(kt == 7),
                                    tile_position=(0, 32 * j),
                                )
                        st = stg.tile([128, F], bf16)
                        nc.vector.tensor_tensor(st[:], ps[:], bt[:], ALU.add)
                        for j in range(4):
                            nc.sync.dma_start(
                                in_b[:, sb, j, :],
                                st[32 * j:32 * (j + 1), :])
                        for q in range(4):
                            stats_front(4 * sb + q)
                    if sb >= 1:
                        for q in range(4):
                            stats_back(4 * (sb - 1) + q)

            # all-to-all: row 4*src+i on this core <- core src's params for
            # this core's local sample i
            nc.gpsimd.collective_compute(
                "AllToAll", ALU.bypass,
                replica_groups=[list(range(NCORES))],
                ins=[in_b.opt()], outs=[out_b.opt()],
            )

            # phase C: per-sample weight norms, then bmm1/silu/bmm2.
            # All ACT Sqrt ops are emitted before any Silu so the activation
            # table loads exactly twice in the whole kernel.
            with ExitStack() as es:
                def pool(name, bufs, space=None):
                    kw = {"space": space} if space else {}
                    return es.enter_context(
                        tc.tile_pool(name=name, bufs=bufs, **kw))
                p_fc1 = pool("fc1", 2)
                p_fc1g = pool("fc1g", SPC)
                p_fc2 = pool("fc2", 4 * SPC)
                p_sq = pool("sq", 2)
                p_rn = pool("rn", 2 * SPC)
                p_tmp = pool("tmp", 4)
                p_h1 = pool("h1", 3)
                p_ob = pool("ob", 3)
                p_pn = pool("pn", 1, "PSUM")
                p_ph1 = pool("ph1", 4, "PSUM")
                p_ph2 = pool("ph2", 2, "PSUM")

                fc1gs, fc2rs, rn1s, rn2s = [], [], [], []
                for i in range(SPC):
                    fc1r = p_fc1.tile([HX, F], bf16)
                    for src in range(4):
                        r = 4 * src + i
                        nc.sync.dma_start(
                            fc1r[32 * src:32 * (src + 1), :],
                            out_b[r:r + 1, :].rearrange(
                                "o (a f) -> (o a) f", a=32),
                        )
                    fc2r = []
                    for fb in range(4):
                        r = 16 + 4 * fb + i
                        t = p_fc2.tile([128, HX], bf16)
                        nc.sync.dma_start(
                            t[:],
                            out_b[r:r + 1, :].rearrange(
                                "o (p q) -> (o p) q", p=128),
                        )
                        fc2r.append(t)

                    # fc1 col norms over d -> rn1 [f_part, fb]
                    sq1 = p_sq.tile([HX, F], bf16)
                    nc.vector.tensor_tensor(sq1[:], fc1r[:], fc1r[:], ALU.mult)
                    pn1 = p_pn.tile([128, 4], fp32, name="pn")
                    for fb in range(4):
                        nc.tensor.matmul(
                            pn1[:, fb:fb + 1],
                            sq1[:, fb * 128:(fb + 1) * 128],
                            ones_col[:],
                            start=True, stop=True,
                        )
                    n1 = p_tmp.tile([128, 4], fp32)
                    nc.scalar.activation(n1[:], pn1[:], AF.Sqrt)
                    n1m = p_tmp.tile([128, 4], fp32)
                    nc.vector.tensor_scalar_max(n1m[:], n1[:], 1e-12)
                    rn1 = p_rn.tile([128, 4], fp32)
                    nc.vector.reciprocal_approx_fast(rn1[:], n1m[:])

                    # fc2 col norms over f -> rn2 [d_part, 1]
                    sq2 = p_sq.tile([128, F], bf16)
                    for fb in range(4):
                        nc.vector.tensor_tensor(
                            sq2[:, fb * 128:(fb + 1) * 128],
                            fc2r[fb][:], fc2r[fb][:], ALU.mult)
                    pn2 = p_pn.tile([128, 4], fp32, name="pn")
                    for fb in range(4):
                        nc.tensor.matmul(
                            pn2[:, 0:1],
                            sq2[:, fb * 128:(fb + 1) * 128],
                            ones_col[:],
                            start=(fb == 0), stop=(fb == 3),
                        )
                    n2 = p_tmp.tile([128, 1], fp32)
                    nc.scalar.activation(n2[:], pn2[:, 0:1], AF.Sqrt)
                    n2m = p_tmp.tile([128, 1], fp32)
                    nc.vector.tensor_scalar_max(n2m[:], n2[:], 1e-12)
                    rn2 = p_rn.tile([128, 1], fp32)
                    nc.vector.reciprocal_approx_fast(rn2[:], n2m[:])

                    # fold rmsnorm weight g into fc1 rows (per-partition d)
                    fc1g = p_fc1g.tile([HX, F], bf16)
                    nc.vector.tensor_scalar_mul(fc1g[:], fc1r[:], g_t[:])

                    fc1gs.append(fc1g)
                    fc2rs.append(fc2r)
                    rn1s.append(rn1)
                    rn2s.append(rn2)

                for i in range(SPC):
                    fc1g, fc2r = fc1gs[i], fc2rs[i]
                    rn1, rn2 = rn1s[i], rn2s[i]
                    for tb in range(TB):
                        xv = xts[i][:, tb * TS:(tb + 1) * TS]
                        xs = xss[i][:, tb * TS:(tb + 1) * TS]
                        ph2 = p_ph2.tile([HX, TS], fp32)
                        for fb in range(4):
                            ph1 = p_ph1.tile([128, TS], fp32)
                            nc.tensor.matmul(
                                ph1[:],
                                fc1g[:, fb * 128:(fb + 1) * 128],
                                xs,
                                start=True, stop=True,
                            )
                            h1 = p_h1.tile([128, TS], bf16)
                            nc.scalar.activation(h1[:], ph1[:], AF.Silu,
                                                 scale=rn1[:, fb:fb + 1])
                            nc.tensor.matmul(
                                ph2[:], fc2r[fb][:], h1[:],
                                start=(fb == 0), stop=(fb == 3),
                            )
                        ob = p_ob.tile([HX, TS], bf16)
                        nc.vector.scalar_tensor_tensor(
                            ob[:], ph2[:], rn2[:], xv, ALU.mult, ALU.add)
                        nc.sync.dma_start(o_d[i, :, tb * TS:(tb + 1) * TS],
                                          ob[:])
    nc.compile()
    return nc


def _prep_inputs(x, s, W, b, g):
    s_p = np.ascontiguousarray(
        s.T.reshape(8, 128, B).transpose(1, 0, 2).reshape(128, 8 * B)
    ).astype(BF16)
    g_p = np.ascontiguousarray(g.reshape(HX, 1))
    Wb = W.astype(BF16)
    xb = x.astype(BF16)
    bb = b.astype(BF16)
    in_maps = []
    for c in range(NCORES):
        Wc = Wb[:, c * COLS:(c + 1) * COLS]
        Wc = np.ascontiguousarray(
            Wc.reshape(8, 128, NB, F).transpose(2, 1, 0, 3)
              .reshape(NB, 128, 8 * F))
        bc = np.ascontiguousarray(np.broadcast_to(
            bb[c * COLS:(c + 1) * COLS].reshape(NB // 4, 4, 1, F),
            (NB // 4, 4, 32, F)).reshape(NB // 4, 128, F))
        xc = np.ascontiguousarray(
            xb[SPC * c:SPC * (c + 1)].transpose(0, 2, 1))
        in_maps.append({"W": Wc, "x": xc, "s": s_p, "b": bc, "g": g_p})
    return in_maps


def kernel(x, s, W, b, g):
    global LAST_EXEC_NS, _cached_nc
    x = np.asarray(x, dtype=np.float32)
    s = np.asarray(s, dtype=np.float32)
    W = np.asarray(W, dtype=np.float32)
    b = np.asarray(b, dtype=np.float32)
    g = np.asarray(g, dtype=np.float32)

    trace = os.environ.get("KERNEL_TRACE", "0") == "1"
    if trace:
        _ensure_axon_hooks()
    if _cached_nc is None:
        _cached_nc = _build()
    in_maps = _prep_inputs(x, s, W, b, g)
    res = run_bass_kernel_spmd(_cached_nc, in_maps, list(range(NCORES)),
                               trace=trace)
    LAST_EXEC_NS = res.exec_time_ns
    out = np.concatenate(
        [np.asarray(res.results[c]["o"]).astype(np.float32)
         for c in range(NCORES)], axis=0)
    return np.ascontiguousarray(out.transpose(0, 2, 1))
